# revision 10
# baseline (speedup 1.0000x reference)
"""Trainium2 Bass kernel for a 2-layer minGRU (B=8, S=4096, D=H=512).

Sharding: data-parallel over batch B across 8 NeuronCores (core b gets x[b]);
weights replicated.

Per-core layout: channels on SBUF partitions, sequence on the free dim.
 - gate preactivations k = W @ x^T computed with PE matmuls (contract over
   input channels, 4 K-chunks of 128), outputs [128h, 512s] in PSUM.
 - a = sigmoid(-(k+bz)) = 1-z        (ACT, bias/scale fused)
 - h~ = max(sigmoid(p+bh), p+bh+0.5) (exact identity for the reference g())
 - b = (1-a) * h~
 - h_t = a_t*h_{t-1} + b_t via DVE tensor_tensor_scan along the free dim,
   chained across sequence tiles via initial=prev[:, -1:].
Layer 2 consumes layer-1's hidden states directly ([H,S] layout is already
the moving-operand layout for the next matmul) and is pipelined behind
layer 1 at sequence-tile granularity.
"""

import numpy as np
import ml_dtypes

import concourse.bass as bass
import concourse.tile as tile
from concourse import bacc, mybir
from concourse.bass_utils import run_bass_kernel_spmd

B, S, D, H, L = 8, 4096, 512, 512, 2
P = 128
HC = H // P       # 4 output-channel chunks
KC = D // P       # 4 contraction chunks
T = 512           # sequence tile
NS = S // T       # 8 sequence tiles

F32 = mybir.dt.float32
F32R = mybir.dt.float32r
BF16 = mybir.dt.bfloat16
AF = mybir.ActivationFunctionType
OP = mybir.AluOpType

N_CORES = 8


def _build():
    nc = bacc.Bacc("TRN2", target_bir_lowering=False, debug=False)

    # ---- DRAM I/O (per core) ----
    xT = nc.dram_tensor("xT", [D, S], BF16, kind="ExternalInput").ap()
    wT = {}
    for l in range(L):
        for g in ("z", "h"):
            wT[(l, g)] = nc.dram_tensor(
                f"w{g}T{l}", [H, H], BF16, kind="ExternalInput"
            ).ap()
    bias_d = {}
    for l in range(L):
        for nm in ("nbz", "bh", "bp"):
            bias_d[(l, nm)] = nc.dram_tensor(
                f"{nm}{l}", [P, HC], F32, kind="ExternalInput"
            ).ap()
    h2T = nc.dram_tensor("h2T", [H, S], BF16, kind="ExternalOutput").ap()
    fin = nc.dram_tensor("fin", [L, H], BF16, kind="ExternalOutput").ap()

    with tile.TileContext(nc) as tc:
        with (
            tc.tile_pool(name="const", bufs=1) as cpool,
            tc.tile_pool(name="xp", bufs=6) as xpool,
            tc.tile_pool(name="h1p", bufs=8) as h1pool,
            tc.tile_pool(name="h2p", bufs=3) as h2pool,
            tc.tile_pool(name="wk", bufs=3) as wk,
            tc.tile_pool(name="ps", bufs=3, space="PSUM") as pp,
        ):
            # ---- weights + biases to SBUF ----
            w_sb = {}
            for l in range(L):
                for g in ("z", "h"):
                    t = cpool.tile([P, KC, H], BF16, tag=f"w{g}{l}")
                    nc.sync.dma_start(
                        t[:], wT[(l, g)].rearrange("(c p) h -> p c h", p=P)
                    )
                    w_sb[(l, g)] = t
            bias_sb = {}
            for (l, nm), ap in bias_d.items():
                t = cpool.tile([P, HC], F32, tag=f"{nm}{l}")
                nc.sync.dma_start(t[:], ap)
                bias_sb[(l, nm)] = t

            # ---- x tiles (per K-chunk, per seq tile) ----
            xt = {}
            for si in range(NS):
                for d in range(KC):
                    t = xpool.tile([P, T], BF16, tag=f"x{d}")
                    nc.sync.dma_start(
                        t[:], xT[d * P : (d + 1) * P, si * T : (si + 1) * T]
                    )
                    xt[(d, si)] = t

            h1_tiles = {}          # (hc, si) -> tile (bf16)
            h2_prev = [None] * HC  # chaining for layer-2 scan

            def emit_layer(l, si):
                for hc in range(HC):
                    kz = pp.tile([P, T], F32, tag="psA")
                    kh = pp.tile([P, T], F32, tag="psB")
                    for g, acc in (("z", kz), ("h", kh)):
                        for d in range(KC):
                            lhsT = w_sb[(l, g)][:, d, hc * P : (hc + 1) * P]
                            if l == 0:
                                rhs = xt[(d, si)][:]
                            else:
                                rhs = h1_tiles[(d, si)][:]
                            nc.tensor.matmul(
                                acc[:], lhsT, rhs,
                                start=(d == 0), stop=(d == KC - 1),
                            )
                    nbz = bias_sb[(l, "nbz")][:, hc : hc + 1]
                    bh = bias_sb[(l, "bh")][:, hc : hc + 1]
                    bp = bias_sb[(l, "bp")][:, hc : hc + 1]

                    a = wk.tile([P, T], BF16, tag="a")
                    nc.scalar.activation(a[:], kz[:], AF.Sigmoid, bias=nbz, scale=-1.0)
                    s = wk.tile([P, T], BF16, tag="s")
                    nc.scalar.activation(s[:], kh[:], AF.Sigmoid, bias=bh)
                    ht = wk.tile([P, T], BF16, tag="ht")
                    # ht = max(kh + (bh+0.5), s)  == g(kh+bh) given s=sigmoid(kh+bh)
                    nc.vector.scalar_tensor_tensor(
                        ht[:], kh[:], bp, s[:], op0=OP.add, op1=OP.max
                    )
                    z = wk.tile([P, T], BF16, tag="z")
                    nc.vector.tensor_scalar(z[:], a[:], -1.0, 1.0, OP.mult, OP.add)
                    b = wk.tile([P, T], BF16, tag="b")
                    nc.vector.tensor_mul(b[:], z[:], ht[:])

                    if l == 0:
                        out_t = h1pool.tile([P, T], BF16, tag=f"h1_{hc}")
                        init = 0.5 if si == 0 else h1_tiles[(hc, si - 1)][:, T - 1 : T]
                        nc.vector.tensor_tensor_scan(
                            out_t[:], a[:], b[:], init, op0=OP.mult, op1=OP.add
                        )
                        h1_tiles[(hc, si)] = out_t
                    else:
                        out_t = h2pool.tile([P, T], BF16, tag=f"h2_{hc}")
                        init = 0.5 if si == 0 else h2_prev[hc][:, T - 1 : T]
                        nc.vector.tensor_tensor_scan(
                            out_t[:], a[:], b[:], init, op0=OP.mult, op1=OP.add
                        )
                        h2_prev[hc] = out_t
                        nc.sync.dma_start(
                            h2T[hc * P : (hc + 1) * P, si * T : (si + 1) * T],
                            out_t[:],
                        )
                        if si == NS - 1:
                            nc.sync.dma_start(
                                fin[1, hc * P : (hc + 1) * P],
                                out_t[:, T - 1 : T],
                            )

            # pipeline: L1(si) then L2(si-1), so PE always has L1 work queued
            # ahead of the L2 group that waits on L1(si)'s scan.
            for si in range(NS):
                emit_layer(0, si)
                if si >= 1:
                    emit_layer(1, si - 1)
            # finals for layer 1 (last column of h1)
            for hc in range(HC):
                nc.sync.dma_start(
                    fin[0, hc * P : (hc + 1) * P],
                    h1_tiles[(hc, NS - 1)][:, T - 1 : T],
                )
            emit_layer(1, NS - 1)

    nc.compile()
    return nc


_nc_cache = None
LAST_RESULTS = None  # BassKernelResults of the most recent run (for test.py)


def _get_nc():
    global _nc_cache
    if _nc_cache is None:
        _nc_cache = _build()
    return _nc_cache


def _chunk_bias(v):
    # (512,) -> [128, 4] where column c is channels [c*128, (c+1)*128)
    return np.ascontiguousarray(v.reshape(HC, P).T.astype(np.float32))


def kernel(x, wz0, bz0, wh0, bh0, wz1, bz1, wh1, bh1, **_):
    x = np.asarray(x, dtype=np.float32)
    nc = _get_nc()

    bf = ml_dtypes.bfloat16
    common = {
        "wzT0": np.ascontiguousarray(np.asarray(wz0).T.astype(bf)),
        "whT0": np.ascontiguousarray(np.asarray(wh0).T.astype(bf)),
        "wzT1": np.ascontiguousarray(np.asarray(wz1).T.astype(bf)),
        "whT1": np.ascontiguousarray(np.asarray(wh1).T.astype(bf)),
        "nbz0": _chunk_bias(-np.asarray(bz0)),
        "bh0": _chunk_bias(np.asarray(bh0)),
        "bp0": _chunk_bias(np.asarray(bh0) + 0.5),
        "nbz1": _chunk_bias(-np.asarray(bz1)),
        "bh1": _chunk_bias(np.asarray(bh1)),
        "bp1": _chunk_bias(np.asarray(bh1) + 0.5),
    }
    in_maps = [
        {**common, "xT": np.ascontiguousarray(x[b].T.astype(bf))} for b in range(B)
    ]
    res = run_bass_kernel_spmd(nc, in_maps, core_ids=list(range(N_CORES)))
    global LAST_RESULTS
    LAST_RESULTS = res

    out = np.empty((B, S, H), np.float32)
    fin = np.empty((L, B, 1, H), np.float32)
    for b in range(B):
        out[b] = res.results[b]["h2T"].astype(np.float32).T
        f = res.results[b]["fin"].astype(np.float32)
        fin[0, b, 0] = f[0]
        fin[1, b, 0] = f[1]
    return out, fin
